# revision 2
# baseline (speedup 1.0000x reference)
"""GatedAttention Trainium2 kernel.

Math (per batch b):
  Qw = x @ Wq + bq            (N, A)
  Kw = x @ Wk + bk            (N, A)
  g  = sigmoid(Qw @ Wv + bv)  (N,)
  S  = Qw @ Kw^T, diag -> -inf
  P  = softmax(S, axis=0)     (column softmax)
  out = (1-g)[:,None] * P + g[:,None] * I

Sharding: 8 cores = 4 batches x 2 column-halves of the score matrix.
Column softmax is independent per column, so no cross-core reduction.

Device layout: scores computed transposed, sT[j, i] tiles (j on partitions)
so the softmax reduction over i is a free-axis reduction. The i axis is
host-permuted so each core's diagonal block sits at i in [0, 2048) —
this keeps the program identical across cores (pure SPMD).

Precision: x / Wq / Wk are host-cast to fp16 (PE rate is the same
1 cycle/row as fp32r but DMA traffic halves and no on-device rounding
casts are needed). Projections accumulate in fp32 PSUM; scores
accumulate in fp32 PSUM; Exp output in bf16. Measured rel L2 err ~1e-3.

Pipeline per core:
  - x fp16 chunks split across both DMA queues; weights on sync.
  - projections: per i-block of 512, QwT/KwT (fp16) accumulated over
    8 H-chunks in fp32 PSUM.
  - gate fused into the projection loop, one iteration deferred so its
    rank-1 broadcast matmuls never head-block ready projection matmuls:
    z = Qw@Wv (PE), 1-g = sigmoid(-z-bv), g = sigmoid(z+bv) on ACT,
    broadcast to [128, N] planes (bf16) via ones-vector rank-1 matmuls.
  - score loop over 16 column tiles: sT chunks in PSUM -> diag(-1e30)
    -> Exp to bf16 (+row sums) -> x(1/denom)x(1-g_i) on DVE (bf16 2x)
    -> +diag(g) -> DMA out in bf16 split across both queues (host
    upcasts).
"""
import numpy as np

import concourse.bacc as bacc
import concourse.mybir as mybir
import concourse.tile as tile
from concourse.bass_utils import run_bass_kernel_spmd

FP32 = mybir.dt.float32
FP16 = mybir.dt.float16
BF16 = mybir.dt.bfloat16
AF = mybir.ActivationFunctionType
ALU = mybir.AluOpType
AX = mybir.AxisListType

B, N, H, A = 4, 4096, 1024, 512
NSH = N // 2          # per-core column shard
NEG = -1.0e30

_CACHE = {}


def _build():
    nc = bacc.Bacc("TRN2", target_bir_lowering=False, debug=False, num_devices=8)
    xq = nc.dram_tensor("xq", [H, N], FP16, kind="ExternalInput").ap()
    wq = nc.dram_tensor("wq", [H, A], FP16, kind="ExternalInput").ap()
    wk = nc.dram_tensor("wk", [H, A], FP16, kind="ExternalInput").ap()
    misc = nc.dram_tensor("misc", [128, 18], FP32, kind="ExternalInput").ap()
    wv16 = nc.dram_tensor("wv16", [128, 8], FP16, kind="ExternalInput").ap()
    eye = nc.dram_tensor("eye", [128, 128], FP32, kind="ExternalInput").ap()
    out = nc.dram_tensor("out", [NSH, N], BF16, kind="ExternalOutput").ap()

    with tile.TileContext(nc) as tc:
        with (
            tc.tile_pool(name="const", bufs=1) as cpool,
            tc.tile_pool(name="proj_out", bufs=1) as qkpool,
            tc.tile_pool(name="bcast", bufs=1) as bcp,
            tc.tile_pool(name="gaterow", bufs=4) as gtmp,
        ):
            # ---- memset-only constants first: the warm-up burst depends
            # only on these, so the PE starts right after the preamble.
            ones_f = cpool.tile([1, 128], FP32, tag="onesf", name="onesf")
            nc.vector.memset(ones_f[:], 1.0)
            ones_h = cpool.tile([1, 128], FP16, tag="ones", name="ones")
            nc.vector.tensor_copy(ones_h[:], ones_f[:])

            # ---- DMA'd constants
            ident = cpool.tile([128, 128], FP32, tag="ident", name="ident")
            nc.sync.dma_start(ident[:], eye)
            misc_sb = cpool.tile([128, 18], FP32, tag="misc", name="misc")
            nc.gpsimd.dma_start(misc_sb[:], misc)
            wv_sb = cpool.tile([128, 8], FP16, tag="wv", name="wv")
            nc.gpsimd.dma_start(wv_sb[:], wv16)
            identb = cpool.tile([128, 128], BF16, tag="identb", name="identb")
            nc.vector.tensor_copy(identb[:], ident[:])
            dneg = cpool.tile([128, 128], FP32, tag="dneg", name="dneg")
            nc.vector.tensor_scalar(dneg[:], ident[:], NEG, None, op0=ALU.mult)

            # ---- persistent projection outputs (fp16) ----
            qwt = [qkpool.tile([128, N], FP16, tag=f"qwt{a}", name=f"qwt{a}")
                   for a in range(4)]
            kwt = [qkpool.tile([128, NSH], FP16, tag=f"kwt{a}", name=f"kwt{a}")
                   for a in range(4)]
            # gate broadcast planes (bf16): g1m[p,i] = 1-g_i, gbc[p,i] = g_i
            g1m_bc = bcp.tile([128, N], BF16, tag="g1mbc", name="g1mbc")
            g_bc = bcp.tile([128, N], BF16, tag="gbc", name="gbc")

            # ---- projections + fused gate ----
            with (
                tc.tile_pool(name="wtiles", bufs=1) as wpool,
                tc.tile_pool(name="xslices", bufs=15) as xpool,
                tc.tile_pool(name="projps", bufs=4, space="PSUM") as ppool,
                tc.tile_pool(name="zrowps", bufs=2, space="PSUM") as zpool,
                tc.tile_pool(name="bcps", bufs=2, space="PSUM") as bps,
            ):
                # PE warm-up: keep the HAM activity monitor busy during the
                # DMA lead-in so the first real matmuls run at full clock.
                # Depends only on the ones memset, so it starts ~3us earlier
                # than an identity-DMA-seeded burst would.
                warm = ppool.tile([128, 512], FP32, tag="ps", name="warm")
                for _ in range(32):
                    nc.tensor.matmul(warm[0:64, 0:64], ones_h[:, 0:64],
                                     ones_h[:, 0:64], start=True, stop=True)

                def load_w(dram, h, lst, tag):
                    wt = wpool.tile([128, A], FP16, tag=f"{tag}{h}",
                                    name=f"{tag}{h}")
                    nc.sync.dma_start(wt[:], dram[h * 128:(h + 1) * 128, :])
                    lst.append(wt)

                wqr, wkr = [], []
                for h in range(8):
                    load_w(wq, h, wqr, "wqr")
                # wk DMAs are emitted after ib0/ib1's x prefetch (cold-start
                # ordering: K matmuls for ib0 run ~25us in, x is needed first)

                grows = [None] * 8  # (g1m_row, g_row) per i-block

                def emit_gate(ib):
                    # z = Qw @ Wv (dup pair cols so both outputs land on
                    # partition 0); 1-g = sigmoid(-z-bv), g = sigmoid(z+bv)
                    pzc = zpool.tile([2, 512], FP32, tag="zr", name="zr")
                    for a in range(4):
                        nc.tensor.matmul(
                            pzc[:], wv_sb[:, 2 * a:2 * a + 2],
                            qwt[a][:, ib * 512:(ib + 1) * 512],
                            start=(a == 0), stop=(a == 3))
                    g1 = gtmp.tile([1, 512], FP16, tag="g1", name="g1")
                    nc.scalar.activation(g1[:], pzc[0:1, :], AF.Sigmoid,
                                         scale=-1.0, bias=misc_sb[0:1, 17:18])
                    g2 = gtmp.tile([1, 512], FP16, tag="g2", name="g2")
                    nc.scalar.activation(g2[:], pzc[0:1, :], AF.Sigmoid,
                                         bias=misc_sb[0:1, 16:17])
                    grows[ib] = (g1, g2)

                def emit_bcast(ib):
                    g1, g2 = grows[ib]
                    sl = slice(ib * 512, (ib + 1) * 512)
                    pb = bps.tile([128, 512], FP32, tag="pb", name="pb")
                    nc.tensor.matmul(pb[:], ones_h[:], g1[:],
                                     start=True, stop=True)
                    nc.vector.tensor_copy(g1m_bc[:, sl], pb[:])
                    pb2 = bps.tile([128, 512], FP32, tag="pb", name="pb")
                    nc.tensor.matmul(pb2[:], ones_h[:], g2[:],
                                     start=True, stop=True)
                    nc.scalar.copy(g_bc[:, sl], pb2[:])

                def load_x(ib, gpsimd_all=False):
                    xs = []
                    for h in range(8):
                        xr = xpool.tile([128, 512], FP16, tag="xr", name="xr")
                        src = xq[h * 128:(h + 1) * 128, ib * 512:(ib + 1) * 512]
                        if h % 2 == 0 and not gpsimd_all:
                            nc.sync.dma_start(xr[:], src)
                        else:
                            nc.gpsimd.dma_start(xr[:], src)
                        xs.append(xr)
                    return xs

                xs_next = None
                for ib in range(8):
                    if ib == 0:
                        # ib0 entirely via gpsimd: the sync queue is busy
                        # with weights
                        xs = load_x(0, gpsimd_all=True)
                        xs_next = load_x(1)
                        for h in range(8):
                            load_w(wk, h, wkr, "wkr")
                    else:
                        xs = xs_next
                        xs_next = load_x(ib + 1) if ib < 7 else None
                    for a in range(4):
                        pq = ppool.tile([128, 512], FP32, tag="ps", name="ps")
                        for h in range(8):
                            nc.tensor.matmul(pq[:], wqr[h][:, a * 128:(a + 1) * 128],
                                             xs[h][:], start=(h == 0), stop=(h == 7))
                        nc.scalar.activation(qwt[a][:, ib * 512:(ib + 1) * 512], pq[:],
                                             AF.Identity, bias=misc_sb[:, a:a + 1])
                        if ib < 4:
                            pk = ppool.tile([128, 512], FP32, tag="ps", name="ps")
                            for h in range(8):
                                nc.tensor.matmul(pk[:], wkr[h][:, a * 128:(a + 1) * 128],
                                                 xs[h][:], start=(h == 0), stop=(h == 7))
                            nc.scalar.activation(kwt[a][:, ib * 512:(ib + 1) * 512],
                                                 pk[:], AF.Identity,
                                                 bias=misc_sb[:, 4 + a:5 + a])
                    # deferred gate pipeline: keeps the PE queue head fed with
                    # ready matmuls (operands of these are >=1 iteration old)
                    if ib >= 1:
                        emit_gate(ib - 1)
                    if ib >= 2:
                        emit_bcast(ib - 2)
                emit_gate(7)
                emit_bcast(6)
                emit_bcast(7)

            # ---- main loop over column tiles (output stays transposed) ----
            with (
                tc.tile_pool(name="expp", bufs=4) as epool,
                tc.tile_pool(name="dsum", bufs=2) as dpool,
                tc.tile_pool(name="diag", bufs=2) as dzpool,
                tc.tile_pool(name="scoreps", bufs=4, space="PSUM") as sps,
            ):
                for t in range(16):
                    exp_t = epool.tile([128, N], BF16, tag="exp", name="exp")
                    dsum = dpool.tile([128, 4], FP32, tag="ds", name="ds")
                    dch = (t * 128) // 1024
                    for ch in range(4):
                        ps = sps.tile([128, 1024], FP32, tag="sc", name="sc")
                        for sub in range(2):
                            o = ch * 1024 + sub * 512
                            for a in range(4):
                                nc.tensor.matmul(ps[:, sub * 512:(sub + 1) * 512],
                                                 kwt[a][:, t * 128:(t + 1) * 128],
                                                 qwt[a][:, o:o + 512],
                                                 start=(a == 0), stop=(a == 3))
                        if ch == dch:
                            off = t * 128 - ch * 1024
                            nc.vector.tensor_add(ps[:, off:off + 128],
                                                 ps[:, off:off + 128], dneg[:])
                        nc.scalar.activation(exp_t[:, ch * 1024:(ch + 1) * 1024],
                                             ps[:], AF.Exp,
                                             accum_out=dsum[:, ch:ch + 1])
                    rcol = dpool.tile([128, 1], FP32, tag="r", name="r")
                    nc.vector.tensor_reduce(rcol[:], dsum[:], axis=AX.X, op=ALU.add)
                    nc.vector.reciprocal(rcol[:], rcol[:])
                    for ch in range(2):
                        sl = slice(ch * 1024, (ch + 1) * 1024)
                        nc.vector.scalar_tensor_tensor(
                            exp_t[:, sl], exp_t[:, sl], rcol[:],
                            g1m_bc[:, sl], op0=ALU.mult, op1=ALU.mult)
                    # diagonal (always lands in the first half: j-local == i)
                    dz = dzpool.tile([128, 128], BF16, tag="dz", name="dz")
                    nc.vector.tensor_mul(dz[:], identb[:],
                                         g_bc[:, t * 128:(t + 1) * 128])
                    nc.vector.tensor_add(exp_t[:, t * 128:(t + 1) * 128],
                                         exp_t[:, t * 128:(t + 1) * 128], dz[:])
                    # first half on one queue, second half on the other, so
                    # each tile's MB drains through both DGE rings in parallel
                    eng, eng2 = ((nc.sync, nc.gpsimd) if t % 2 == 1
                                 else (nc.gpsimd, nc.sync))
                    eng.dma_start(out[t * 128:(t + 1) * 128, 0:2048],
                                  exp_t[:, 0:2048])
                    for ch in range(2, 4):
                        sl = slice(ch * 1024, (ch + 1) * 1024)
                        nc.vector.scalar_tensor_tensor(
                            exp_t[:, sl], exp_t[:, sl], rcol[:],
                            g1m_bc[:, sl], op0=ALU.mult, op1=ALU.mult)
                        if t == 15:
                            # final tile: quarter DMAs to shorten the tail
                            e = eng if ch == 2 else eng2
                            e.dma_start(out[t * 128:(t + 1) * 128, sl],
                                        exp_t[:, sl])
                    if t < 15:
                        eng2.dma_start(out[t * 128:(t + 1) * 128, 2048:4096],
                                       exp_t[:, 2048:4096])
    nc.compile()
    return nc


def kernel(x, Wq, bq, Wk, bk, Wv, bv, _trace=False, _tmpdir=None):
    x = np.asarray(x, dtype=np.float32)
    if "nc" not in _CACHE:
        _CACHE["nc"] = _build()
    nc = _CACHE["nc"]

    bv_f = np.float32(np.asarray(bv).reshape(())[()])
    eye_np = np.eye(128, dtype=np.float32)
    misc = np.zeros((128, 18), dtype=np.float32)
    misc[:, 0:4] = np.asarray(bq, np.float32).reshape(4, 128).T
    misc[:, 4:8] = np.asarray(bk, np.float32).reshape(4, 128).T
    misc[:, 16] = bv_f
    misc[:, 17] = -bv_f
    wv16 = np.zeros((128, 8), dtype=np.float16)
    wv_c = np.asarray(Wv, np.float32).reshape(4, 128).T
    wv16[:, 0:8:2] = wv_c
    wv16[:, 1:8:2] = wv_c
    wq_np = np.ascontiguousarray(np.asarray(Wq, np.float32).astype(np.float16))
    wk_np = np.ascontiguousarray(np.asarray(Wk, np.float32).astype(np.float16))

    in_maps = []
    for c in range(8):
        b, h = c // 2, c % 2
        xT = x[b].T.astype(np.float16)  # (H, N)
        if h == 0:
            xqc = np.ascontiguousarray(xT)
        else:
            xqc = np.ascontiguousarray(
                np.concatenate([xT[:, NSH:], xT[:, :NSH]], axis=1))
        in_maps.append({"xq": xqc, "wq": wq_np, "wk": wk_np, "misc": misc,
                        "wv16": wv16, "eye": eye_np})

    res = run_bass_kernel_spmd(nc, in_maps, list(range(8)), trace=_trace,
                               tmpdir=_tmpdir)

    outp = np.empty((B, N, N), dtype=np.float32)
    for c in range(8):
        b, h = c // 2, c % 2
        O = np.asarray(res.results[c]["out"]).astype(np.float32).T  # (i_perm, j)
        js = slice(h * NSH, (h + 1) * NSH)
        outp[b, h * NSH:(h + 1) * NSH, js] = O[:NSH]
        outp[b, (1 - h) * NSH:(2 - h) * NSH, js] = O[NSH:]
    if _trace:
        return outp, res
    return outp


# revision 3
# speedup vs baseline: 1.0897x; 1.0897x over previous
"""GatedAttention Trainium2 kernel.

Math (per batch b):
  Qw = x @ Wq + bq            (N, A)
  Kw = x @ Wk + bk            (N, A)
  g  = sigmoid(Qw @ Wv + bv)  (N,)
  S  = Qw @ Kw^T, diag -> -inf
  P  = softmax(S, axis=0)     (column softmax)
  out = (1-g)[:,None] * P + g[:,None] * I

Sharding: 8 cores = 4 batches x 2 column-halves of the score matrix.
Column softmax is independent per column, so no cross-core reduction.

Device layout: scores computed transposed, sT[j, i] tiles (j on partitions)
so the softmax reduction over i is a free-axis reduction. The i axis is
host-permuted so each core's diagonal block sits at i in [0, 2048) —
this keeps the program identical across cores (pure SPMD).

Dtypes: x / Wq / Wk ship as fp16 (half the HBM read traffic) and are
upcast on-device to fp32r, which streams through the PE at 227ns per
512-row matmul — measurably faster than fp16/bf16 operands (259ns).
Projections/scores accumulate in fp32 PSUM; Exp output in bf16.

Pipeline per core:
  - x fp16: odd H-chunks via gpsimd SWDGE (casts fp16->fp32r inline),
    even chunks staged fp16 on sync HWDGE + DVE upcast; weights on sync.
  - projections: per i-block of 512, QwT/KwT accumulated over 8 H-chunks.
  - gate fused into the projection loop, one iteration deferred so its
    rank-1 broadcast matmuls never head-block ready projection matmuls:
    z = Qw@Wv (PE), 1-g = sigmoid(-z-bv), g = sigmoid(z+bv) on ACT,
    broadcast to [128, N] planes (bf16) via ones-vector rank-1 matmuls.
  - score loop over 16 column tiles: sT chunks in PSUM -> diag(-1e30)
    -> Exp to bf16 (+row sums) -> x(1/denom)x(1-g_i) on DVE (bf16 2x)
    -> +diag(g) -> DMA out in bf16 split across both queues (host
    upcasts).
"""
import numpy as np

import concourse.bacc as bacc
import concourse.mybir as mybir
import concourse.tile as tile
from concourse.bass_utils import run_bass_kernel_spmd

FP32 = mybir.dt.float32
FP32R = mybir.dt.float32r
FP16 = mybir.dt.float16
BF16 = mybir.dt.bfloat16
AF = mybir.ActivationFunctionType
ALU = mybir.AluOpType
AX = mybir.AxisListType

B, N, H, A = 4, 4096, 1024, 512
NSH = N // 2          # per-core column shard
NEG = -1.0e30

_CACHE = {}


def _build():
    nc = bacc.Bacc("TRN2", target_bir_lowering=False, debug=False, num_devices=8)
    xq = nc.dram_tensor("xq", [H, N], FP16, kind="ExternalInput").ap()
    wq = nc.dram_tensor("wq", [H, A], FP16, kind="ExternalInput").ap()
    wk = nc.dram_tensor("wk", [H, A], FP16, kind="ExternalInput").ap()
    misc = nc.dram_tensor("misc", [128, 18], FP32, kind="ExternalInput").ap()
    eye = nc.dram_tensor("eye", [128, 128], FP32, kind="ExternalInput").ap()
    out = nc.dram_tensor("out", [NSH, N], BF16, kind="ExternalOutput").ap()

    with tile.TileContext(nc) as tc:
        with (
            tc.tile_pool(name="const", bufs=1) as cpool,
            tc.tile_pool(name="proj_out", bufs=1) as qkpool,
            tc.tile_pool(name="bcast", bufs=1) as bcp,
            tc.tile_pool(name="gaterow", bufs=4) as gtmp,
        ):
            # ---- memset-only constants first: the warm-up burst depends
            # only on these, so the PE starts right after the preamble.
            ones_f = cpool.tile([1, 128], FP32, tag="onesf", name="onesf")
            nc.vector.memset(ones_f[:], 1.0)
            ones_r = cpool.tile([1, 128], FP32R, tag="ones", name="ones")
            nc.vector.tensor_copy(ones_r[:], ones_f[:])

            # ---- DMA'd constants
            ident = cpool.tile([128, 128], FP32, tag="ident", name="ident")
            nc.sync.dma_start(ident[:], eye)
            misc_sb = cpool.tile([128, 18], FP32, tag="misc", name="misc")
            nc.gpsimd.dma_start(misc_sb[:], misc)
            identb = cpool.tile([128, 128], BF16, tag="identb", name="identb")
            nc.vector.tensor_copy(identb[:], ident[:])
            dneg = cpool.tile([128, 128], FP32, tag="dneg", name="dneg")
            nc.vector.tensor_scalar(dneg[:], ident[:], NEG, None, op0=ALU.mult)
            misc_r = cpool.tile([128, 18], FP32R, tag="miscr", name="miscr")
            nc.vector.tensor_copy(misc_r[:], misc_sb[:])

            # ---- persistent projection outputs (fp32r) ----
            qwt = [qkpool.tile([128, N], FP32R, tag=f"qwt{a}", name=f"qwt{a}")
                   for a in range(4)]
            kwt = [qkpool.tile([128, NSH], FP32R, tag=f"kwt{a}", name=f"kwt{a}")
                   for a in range(4)]
            # gate broadcast planes (bf16): g1m[p,i] = 1-g_i, gbc[p,i] = g_i
            g1m_bc = bcp.tile([128, N], BF16, tag="g1mbc", name="g1mbc")
            g_bc = bcp.tile([128, N], BF16, tag="gbc", name="gbc")

            # ---- projections + fused gate ----
            with (
                tc.tile_pool(name="wtiles", bufs=1) as wpool,
                tc.tile_pool(name="wstage", bufs=3) as wst,
                tc.tile_pool(name="xstage", bufs=4) as xst,
                tc.tile_pool(name="xslices", bufs=15) as xpool,
                tc.tile_pool(name="projps", bufs=4, space="PSUM") as ppool,
                tc.tile_pool(name="zrowps", bufs=2, space="PSUM") as zpool,
                tc.tile_pool(name="bcps", bufs=2, space="PSUM") as bps,
            ):
                # PE warm-up: keep the HAM activity monitor busy during the
                # DMA lead-in so the first real matmuls run at full clock.
                # Depends only on the ones memset: starts right after the
                # framework preamble.
                warm = ppool.tile([128, 512], FP32, tag="ps", name="warm")
                for _ in range(32):
                    nc.tensor.matmul(warm[0:64, 0:64], ones_r[:, 0:64],
                                     ones_r[:, 0:64], start=True, stop=True)

                def load_w(dram, h, lst, tag):
                    wt = wst.tile([128, A], FP16, tag="wst", name="wst")
                    nc.sync.dma_start(wt[:], dram[h * 128:(h + 1) * 128, :])
                    wr = wpool.tile([128, A], FP32R, tag=f"{tag}{h}", name=f"{tag}{h}")
                    nc.vector.tensor_copy(wr[:], wt[:])
                    lst.append(wr)

                wqr, wkr = [], []
                for h in range(8):
                    load_w(wq, h, wqr, "wqr")
                # wk DMAs are emitted after ib0/ib1's x prefetch (cold-start
                # ordering: K matmuls for ib0 run ~25us in, x is needed first)

                grows = [None] * 8  # (g1m_row, g_row) per i-block

                def emit_gate(ib):
                    # z = Qw @ Wv (dup pair cols so both outputs land on
                    # partition 0); 1-g = sigmoid(-z-bv), g = sigmoid(z+bv)
                    pzc = zpool.tile([2, 512], FP32, tag="zr", name="zr")
                    for a in range(4):
                        nc.tensor.matmul(
                            pzc[:], misc_r[:, 8 + 2 * a:10 + 2 * a],
                            qwt[a][:, ib * 512:(ib + 1) * 512],
                            start=(a == 0), stop=(a == 3))
                    g1 = gtmp.tile([1, 512], FP32R, tag="g1", name="g1")
                    nc.scalar.activation(g1[:], pzc[0:1, :], AF.Sigmoid,
                                         scale=-1.0, bias=misc_sb[0:1, 17:18])
                    g2 = gtmp.tile([1, 512], FP32R, tag="g2", name="g2")
                    nc.scalar.activation(g2[:], pzc[0:1, :], AF.Sigmoid,
                                         bias=misc_sb[0:1, 16:17])
                    grows[ib] = (g1, g2)

                def emit_bcast(ib):
                    g1, g2 = grows[ib]
                    sl = slice(ib * 512, (ib + 1) * 512)
                    pb = bps.tile([128, 512], FP32, tag="pb", name="pb")
                    nc.tensor.matmul(pb[:], ones_r[0:1, :], g1[:],
                                     start=True, stop=True)
                    nc.vector.tensor_copy(g1m_bc[:, sl], pb[:])
                    pb2 = bps.tile([128, 512], FP32, tag="pb", name="pb")
                    nc.tensor.matmul(pb2[:], ones_r[0:1, :], g2[:],
                                     start=True, stop=True)
                    nc.scalar.copy(g_bc[:, sl], pb2[:])

                def load_x(ib, gpsimd_all=False):
                    xs = []
                    for h in range(8):
                        xr = xpool.tile([128, 512], FP32R, tag="xr", name="xr")
                        src = xq[h * 128:(h + 1) * 128, ib * 512:(ib + 1) * 512]
                        if h % 2 == 0 and not gpsimd_all:
                            # HWDGE queue (no cast) -> fp16 staging + DVE upcast
                            xt = xst.tile([128, 512], FP16, tag="xst", name="xst")
                            nc.sync.dma_start(xt[:], src)
                            nc.vector.tensor_copy(xr[:], xt[:])
                        else:
                            # SWDGE casts fp16 -> fp32r inline
                            nc.gpsimd.dma_start(xr[:], src)
                        xs.append(xr)
                    return xs

                xs_next = None
                for ib in range(8):
                    if ib == 0:
                        # ib0 entirely via gpsimd: the sync queue is busy with
                        # weights, and SWDGE casts inline (no DVE dependency)
                        xs = load_x(0, gpsimd_all=True)
                        xs_next = load_x(1)
                        for h in range(8):
                            load_w(wk, h, wkr, "wkr")
                    else:
                        xs = xs_next
                        xs_next = load_x(ib + 1) if ib < 7 else None
                    for a in range(4):
                        pq = ppool.tile([128, 512], FP32, tag="ps", name="ps")
                        for h in range(8):
                            nc.tensor.matmul(pq[:], wqr[h][:, a * 128:(a + 1) * 128],
                                             xs[h][:], start=(h == 0), stop=(h == 7))
                        nc.scalar.activation(qwt[a][:, ib * 512:(ib + 1) * 512], pq[:],
                                             AF.Identity, bias=misc_sb[:, a:a + 1])
                        if ib < 4:
                            pk = ppool.tile([128, 512], FP32, tag="ps", name="ps")
                            for h in range(8):
                                nc.tensor.matmul(pk[:], wkr[h][:, a * 128:(a + 1) * 128],
                                                 xs[h][:], start=(h == 0), stop=(h == 7))
                            nc.scalar.activation(kwt[a][:, ib * 512:(ib + 1) * 512],
                                                 pk[:], AF.Identity,
                                                 bias=misc_sb[:, 4 + a:5 + a])
                    # deferred gate pipeline: keeps the PE queue head fed with
                    # ready matmuls (operands of these are >=1 iteration old)
                    if ib >= 1:
                        emit_gate(ib - 1)
                    if ib >= 2:
                        emit_bcast(ib - 2)
                emit_gate(7)
                emit_bcast(6)
                emit_bcast(7)

            # ---- main loop over column tiles (output stays transposed) ----
            with (
                tc.tile_pool(name="expp", bufs=4) as epool,
                tc.tile_pool(name="dsum", bufs=2) as dpool,
                tc.tile_pool(name="diag", bufs=2) as dzpool,
                tc.tile_pool(name="scoreps", bufs=4, space="PSUM") as sps,
            ):
                for t in range(16):
                    exp_t = epool.tile([128, N], BF16, tag="exp", name="exp")
                    dsum = dpool.tile([128, 4], FP32, tag="ds", name="ds")
                    dch = (t * 128) // 1024
                    for ch in range(4):
                        ps = sps.tile([128, 1024], FP32, tag="sc", name="sc")
                        for sub in range(2):
                            o = ch * 1024 + sub * 512
                            for a in range(4):
                                nc.tensor.matmul(ps[:, sub * 512:(sub + 1) * 512],
                                                 kwt[a][:, t * 128:(t + 1) * 128],
                                                 qwt[a][:, o:o + 512],
                                                 start=(a == 0), stop=(a == 3))
                        if ch == dch:
                            off = t * 128 - ch * 1024
                            nc.vector.tensor_add(ps[:, off:off + 128],
                                                 ps[:, off:off + 128], dneg[:])
                        nc.scalar.activation(exp_t[:, ch * 1024:(ch + 1) * 1024],
                                             ps[:], AF.Exp,
                                             accum_out=dsum[:, ch:ch + 1])
                    rcol = dpool.tile([128, 1], FP32, tag="r", name="r")
                    nc.vector.tensor_reduce(rcol[:], dsum[:], axis=AX.X, op=ALU.add)
                    nc.vector.reciprocal(rcol[:], rcol[:])
                    for ch in range(2):
                        sl = slice(ch * 1024, (ch + 1) * 1024)
                        nc.vector.scalar_tensor_tensor(
                            exp_t[:, sl], exp_t[:, sl], rcol[:],
                            g1m_bc[:, sl], op0=ALU.mult, op1=ALU.mult)
                    # diagonal (always lands in the first half: j-local == i)
                    dz = dzpool.tile([128, 128], BF16, tag="dz", name="dz")
                    nc.vector.tensor_mul(dz[:], identb[:],
                                         g_bc[:, t * 128:(t + 1) * 128])
                    nc.vector.tensor_add(exp_t[:, t * 128:(t + 1) * 128],
                                         exp_t[:, t * 128:(t + 1) * 128], dz[:])
                    # first half on one queue, second half on the other, so
                    # each tile's MB drains through both DGE rings in parallel
                    eng, eng2 = ((nc.sync, nc.gpsimd) if t % 2 == 1
                                 else (nc.gpsimd, nc.sync))
                    eng.dma_start(out[t * 128:(t + 1) * 128, 0:2048],
                                  exp_t[:, 0:2048])
                    for ch in range(2, 4):
                        sl = slice(ch * 1024, (ch + 1) * 1024)
                        nc.vector.scalar_tensor_tensor(
                            exp_t[:, sl], exp_t[:, sl], rcol[:],
                            g1m_bc[:, sl], op0=ALU.mult, op1=ALU.mult)
                        if t == 15:
                            # final tile: quarter DMAs to shorten the tail
                            e = eng if ch == 2 else eng2
                            e.dma_start(out[t * 128:(t + 1) * 128, sl],
                                        exp_t[:, sl])
                    if t < 15:
                        eng2.dma_start(out[t * 128:(t + 1) * 128, 2048:4096],
                                       exp_t[:, 2048:4096])
    nc.compile()
    return nc


def kernel(x, Wq, bq, Wk, bk, Wv, bv, _trace=False, _tmpdir=None):
    x = np.asarray(x, dtype=np.float32)
    if "nc" not in _CACHE:
        _CACHE["nc"] = _build()
    nc = _CACHE["nc"]

    bv_f = np.float32(np.asarray(bv).reshape(())[()])
    eye_np = np.eye(128, dtype=np.float32)
    misc = np.zeros((128, 18), dtype=np.float32)
    misc[:, 0:4] = np.asarray(bq, np.float32).reshape(4, 128).T
    misc[:, 4:8] = np.asarray(bk, np.float32).reshape(4, 128).T
    wv_c = np.asarray(Wv, np.float32).reshape(4, 128).T
    misc[:, 8:16:2] = wv_c
    misc[:, 9:16:2] = wv_c
    misc[:, 16] = bv_f
    misc[:, 17] = -bv_f
    wq_np = np.ascontiguousarray(np.asarray(Wq, np.float32).astype(np.float16))
    wk_np = np.ascontiguousarray(np.asarray(Wk, np.float32).astype(np.float16))

    in_maps = []
    for c in range(8):
        b, h = c // 2, c % 2
        xT = x[b].T.astype(np.float16)  # (H, N)
        if h == 0:
            xqc = np.ascontiguousarray(xT)
        else:
            xqc = np.ascontiguousarray(
                np.concatenate([xT[:, NSH:], xT[:, :NSH]], axis=1))
        in_maps.append({"xq": xqc, "wq": wq_np, "wk": wk_np, "misc": misc,
                        "eye": eye_np})

    res = run_bass_kernel_spmd(nc, in_maps, list(range(8)), trace=_trace,
                               tmpdir=_tmpdir)

    outp = np.empty((B, N, N), dtype=np.float32)
    for c in range(8):
        b, h = c // 2, c % 2
        O = np.asarray(res.results[c]["out"]).astype(np.float32).T  # (i_perm, j)
        js = slice(h * NSH, (h + 1) * NSH)
        outp[b, h * NSH:(h + 1) * NSH, js] = O[:NSH]
        outp[b, (1 - h) * NSH:(2 - h) * NSH, js] = O[NSH:]
    if _trace:
        return outp, res
    return outp


# revision 6
# speedup vs baseline: 1.1129x; 1.0213x over previous
"""GatedAttention Trainium2 kernel.

Math (per batch b):
  Qw = x @ Wq + bq            (N, A)
  Kw = x @ Wk + bk            (N, A)
  g  = sigmoid(Qw @ Wv + bv)  (N,)
  S  = Qw @ Kw^T, diag -> -inf
  P  = softmax(S, axis=0)     (column softmax)
  out = (1-g)[:,None] * P + g[:,None] * I

Sharding: 8 cores = 4 batches x 2 column-halves of the score matrix.
Column softmax is independent per column, so no cross-core reduction.

Device layout: scores computed transposed, sT[j, i] tiles (j on partitions)
so the softmax reduction over i is a free-axis reduction. The i axis is
host-permuted so each core's diagonal block sits at i in [0, 2048) —
this keeps the program identical across cores (pure SPMD).

Dtypes: x / Wq / Wk ship as fp16 (half the HBM read traffic) and are
upcast on-device to fp32r, which streams through the PE at 227ns per
512-row matmul — measurably faster than fp16/bf16 operands (259ns).
Projections/scores accumulate in fp32 PSUM; Exp output in bf16.

Pipeline per core:
  - x fp16: odd H-chunks via gpsimd SWDGE (casts fp16->fp32r inline),
    even chunks staged fp16 on sync HWDGE + DVE upcast; weights on sync.
  - projections: per i-block of 512, QwT/KwT accumulated over 8 H-chunks.
  - gate fused into the projection loop, one iteration deferred so its
    rank-1 broadcast matmuls never head-block ready projection matmuls:
    z = Qw@Wv (PE), 1-g = sigmoid(-z-bv), g = sigmoid(z+bv) on ACT,
    broadcast to [128, N] planes (bf16) via ones-vector rank-1 matmuls.
  - score loop over 16 column tiles: sT chunks in PSUM -> diag(-1e30)
    -> Exp to bf16 (+row sums) -> x(1/denom)x(1-g_i) on DVE (bf16 2x)
    -> +diag(g) -> DMA out in bf16 split across both queues (host
    upcasts).
"""
import numpy as np

import concourse.bacc as bacc
import concourse.mybir as mybir
import concourse.tile as tile
from concourse.bass_utils import run_bass_kernel_spmd

FP32 = mybir.dt.float32
FP32R = mybir.dt.float32r
FP16 = mybir.dt.float16
BF16 = mybir.dt.bfloat16
AF = mybir.ActivationFunctionType
ALU = mybir.AluOpType
AX = mybir.AxisListType

B, N, H, A = 4, 4096, 1024, 512
NSH = N // 2          # per-core column shard
NEG = -1.0e30

_CACHE = {}


def _build():
    nc = bacc.Bacc("TRN2", target_bir_lowering=False, debug=False, num_devices=8)
    xq = nc.dram_tensor("xq", [H, N], FP16, kind="ExternalInput").ap()
    wq = nc.dram_tensor("wq", [H, A], FP16, kind="ExternalInput").ap()
    wk = nc.dram_tensor("wk", [H, A], FP16, kind="ExternalInput").ap()
    misc = nc.dram_tensor("misc", [128, 18], FP32, kind="ExternalInput").ap()
    eye = nc.dram_tensor("eye", [128, 128], FP32, kind="ExternalInput").ap()
    out = nc.dram_tensor("out", [NSH, N], BF16, kind="ExternalOutput").ap()

    with tile.TileContext(nc) as tc:
        with (
            tc.tile_pool(name="const", bufs=1) as cpool,
            tc.tile_pool(name="proj_out", bufs=1) as qkpool,
            tc.tile_pool(name="bcast", bufs=1) as bcp,
            tc.tile_pool(name="gaterow", bufs=4) as gtmp,
        ):
            # ---- memset-only constants first: the warm-up burst depends
            # only on these, so the PE starts right after the preamble.
            ones_f = cpool.tile([1, 128], FP32, tag="onesf", name="onesf")
            nc.vector.memset(ones_f[:], 1.0)
            ones_r = cpool.tile([1, 128], FP32R, tag="ones", name="ones")
            nc.vector.tensor_copy(ones_r[:], ones_f[:])

            # ---- DMA'd constants
            ident = cpool.tile([128, 128], FP32, tag="ident", name="ident")
            nc.sync.dma_start(ident[:], eye)
            misc_sb = cpool.tile([128, 18], FP32, tag="misc", name="misc")
            nc.gpsimd.dma_start(misc_sb[:], misc)
            identb = cpool.tile([128, 128], BF16, tag="identb", name="identb")
            nc.vector.tensor_copy(identb[:], ident[:])
            dneg = cpool.tile([128, 128], FP32, tag="dneg", name="dneg")
            nc.vector.tensor_scalar(dneg[:], ident[:], NEG, None, op0=ALU.mult)
            misc_r = cpool.tile([128, 18], FP32R, tag="miscr", name="miscr")
            nc.vector.tensor_copy(misc_r[:], misc_sb[:])

            # ---- persistent projection outputs (fp32r) ----
            qwt = [qkpool.tile([128, N], FP32R, tag=f"qwt{a}", name=f"qwt{a}")
                   for a in range(4)]
            kwt = [qkpool.tile([128, NSH], FP32R, tag=f"kwt{a}", name=f"kwt{a}")
                   for a in range(4)]
            # gate broadcast planes (bf16): g1m[p,i] = 1-g_i, gbc[p,i] = g_i
            g1m_bc = bcp.tile([128, N], BF16, tag="g1mbc", name="g1mbc")
            g_bc = bcp.tile([128, N], BF16, tag="gbc", name="gbc")

            # ---- projections + fused gate ----
            with (
                tc.tile_pool(name="wtiles", bufs=1) as wpool,
                tc.tile_pool(name="wstage", bufs=3) as wst,
                tc.tile_pool(name="xstage", bufs=10) as xst,
                tc.tile_pool(name="xslices", bufs=15) as xpool,
                tc.tile_pool(name="projps", bufs=4, space="PSUM") as ppool,
                tc.tile_pool(name="zrowps", bufs=2, space="PSUM") as zpool,
                tc.tile_pool(name="bcps", bufs=2, space="PSUM") as bps,
            ):
                # PE warm-up: keep the HAM activity monitor busy during the
                # DMA lead-in so the first real matmuls run at full clock.
                # Depends only on the ones memset: starts right after the
                # framework preamble.
                warm = ppool.tile([128, 512], FP32, tag="ps", name="warm")
                for _ in range(32):
                    nc.tensor.matmul(warm[0:64, 0:64], ones_r[:, 0:64],
                                     ones_r[:, 0:64], start=True, stop=True)

                def load_w(dram, h, lst, tag):
                    wt = wst.tile([128, A], FP16, tag="wst", name="wst")
                    nc.sync.dma_start(wt[:], dram[h * 128:(h + 1) * 128, :])
                    wr = wpool.tile([128, A], FP32R, tag=f"{tag}{h}", name=f"{tag}{h}")
                    nc.vector.tensor_copy(wr[:], wt[:])
                    lst.append(wr)

                wqr, wkr = [], []
                for h in range(8):
                    load_w(wq, h, wqr, "wqr")
                # wk DMAs are emitted after ib0/ib1's x prefetch (cold-start
                # ordering: K matmuls for ib0 run ~25us in, x is needed first)

                grows = [None] * 8  # (g1m_row, g_row) per i-block

                def emit_gate(ib):
                    # z = Qw @ Wv (dup pair cols so both outputs land on
                    # partition 0); 1-g = sigmoid(-z-bv), g = sigmoid(z+bv)
                    pzc = zpool.tile([2, 512], FP32, tag="zr", name="zr")
                    for a in range(4):
                        nc.tensor.matmul(
                            pzc[:], misc_r[:, 8 + 2 * a:10 + 2 * a],
                            qwt[a][:, ib * 512:(ib + 1) * 512],
                            start=(a == 0), stop=(a == 3))
                    g1 = gtmp.tile([1, 512], FP32R, tag="g1", name="g1")
                    nc.scalar.activation(g1[:], pzc[0:1, :], AF.Sigmoid,
                                         scale=-1.0, bias=misc_sb[0:1, 17:18])
                    g2 = gtmp.tile([1, 512], FP32R, tag="g2", name="g2")
                    nc.scalar.activation(g2[:], pzc[0:1, :], AF.Sigmoid,
                                         bias=misc_sb[0:1, 16:17])
                    grows[ib] = (g1, g2)

                def emit_bcast(ib):
                    g1, g2 = grows[ib]
                    sl = slice(ib * 512, (ib + 1) * 512)
                    pb = bps.tile([128, 512], FP32, tag="pb", name="pb")
                    nc.tensor.matmul(pb[:], ones_r[0:1, :], g1[:],
                                     start=True, stop=True)
                    nc.vector.tensor_copy(g1m_bc[:, sl], pb[:])
                    pb2 = bps.tile([128, 512], FP32, tag="pb", name="pb")
                    nc.tensor.matmul(pb2[:], ones_r[0:1, :], g2[:],
                                     start=True, stop=True)
                    nc.scalar.copy(g_bc[:, sl], pb2[:])

                def load_x(ib, gpsimd_all=False):
                    # plain fp16 DMAs (SWDGE casting DMAs run ~5x slower than
                    # non-casting ones - the conversion is done by DGE ucode),
                    # then explicit DVE upcasts to fp32r.
                    xs = []
                    for h in range(8):
                        xt = xst.tile([128, 512], FP16, tag="xst", name="xst")
                        src = xq[h * 128:(h + 1) * 128, ib * 512:(ib + 1) * 512]
                        if h % 2 == 0 and not gpsimd_all:
                            nc.sync.dma_start(xt[:], src)
                        else:
                            nc.gpsimd.dma_start(xt[:], src)
                        xr = xpool.tile([128, 512], FP32R, tag="xr", name="xr")
                        nc.vector.tensor_copy(xr[:], xt[:])
                        xs.append(xr)
                    return xs

                xs_next = None
                for ib in range(8):
                    if ib == 0:
                        # ib0 entirely via gpsimd: the sync queue is busy with
                        # weights, and SWDGE casts inline (no DVE dependency)
                        xs = load_x(0, gpsimd_all=True)
                        xs_next = load_x(1)
                        for h in range(8):
                            load_w(wk, h, wkr, "wkr")
                    else:
                        xs = xs_next
                        xs_next = load_x(ib + 1) if ib < 7 else None
                    for a in range(4):
                        pq = ppool.tile([128, 512], FP32, tag="ps", name="ps")
                        for h in range(8):
                            nc.tensor.matmul(pq[:], wqr[h][:, a * 128:(a + 1) * 128],
                                             xs[h][:], start=(h == 0), stop=(h == 7))
                        nc.scalar.activation(qwt[a][:, ib * 512:(ib + 1) * 512], pq[:],
                                             AF.Identity, bias=misc_sb[:, a:a + 1])
                        if ib < 4:
                            pk = ppool.tile([128, 512], FP32, tag="ps", name="ps")
                            for h in range(8):
                                nc.tensor.matmul(pk[:], wkr[h][:, a * 128:(a + 1) * 128],
                                                 xs[h][:], start=(h == 0), stop=(h == 7))
                            nc.scalar.activation(kwt[a][:, ib * 512:(ib + 1) * 512],
                                                 pk[:], AF.Identity,
                                                 bias=misc_sb[:, 4 + a:5 + a])
                    # deferred gate pipeline: keeps the PE queue head fed with
                    # ready matmuls (operands of these are >=1 iteration old)
                    if ib >= 1:
                        emit_gate(ib - 1)
                    if ib >= 2:
                        emit_bcast(ib - 2)
                emit_gate(7)
                emit_bcast(6)
                emit_bcast(7)

            # ---- main loop over column tiles (output stays transposed) ----
            with (
                tc.tile_pool(name="expp", bufs=3) as epool,
                tc.tile_pool(name="dsum", bufs=2) as dpool,
                tc.tile_pool(name="diag", bufs=2) as dzpool,
                tc.tile_pool(name="scoreps", bufs=4, space="PSUM") as sps,
            ):
                for t in range(16):
                    exp_t = epool.tile([128, N], BF16, tag="exp", name="exp")
                    dsum = dpool.tile([128, 4], FP32, tag="ds", name="ds")
                    dch = (t * 128) // 1024
                    for ch in range(4):
                        ps = sps.tile([128, 1024], FP32, tag="sc", name="sc")
                        for sub in range(2):
                            o = ch * 1024 + sub * 512
                            for a in range(4):
                                nc.tensor.matmul(ps[:, sub * 512:(sub + 1) * 512],
                                                 kwt[a][:, t * 128:(t + 1) * 128],
                                                 qwt[a][:, o:o + 512],
                                                 start=(a == 0), stop=(a == 3))
                        if ch == dch:
                            off = t * 128 - ch * 1024
                            nc.vector.tensor_add(ps[:, off:off + 128],
                                                 ps[:, off:off + 128], dneg[:])
                        nc.scalar.activation(exp_t[:, ch * 1024:(ch + 1) * 1024],
                                             ps[:], AF.Exp,
                                             accum_out=dsum[:, ch:ch + 1])
                    rcol = dpool.tile([128, 1], FP32, tag="r", name="r")
                    nc.vector.tensor_reduce(rcol[:], dsum[:], axis=AX.X, op=ALU.add)
                    nc.vector.reciprocal(rcol[:], rcol[:])
                    for ch in range(2):
                        sl = slice(ch * 1024, (ch + 1) * 1024)
                        nc.vector.scalar_tensor_tensor(
                            exp_t[:, sl], exp_t[:, sl], rcol[:],
                            g1m_bc[:, sl], op0=ALU.mult, op1=ALU.mult)
                    # diagonal (always lands in the first half: j-local == i)
                    dz = dzpool.tile([128, 128], BF16, tag="dz", name="dz")
                    nc.vector.tensor_mul(dz[:], identb[:],
                                         g_bc[:, t * 128:(t + 1) * 128])
                    nc.vector.tensor_add(exp_t[:, t * 128:(t + 1) * 128],
                                         exp_t[:, t * 128:(t + 1) * 128], dz[:])
                    # first half on one queue, second half on the other, so
                    # each tile's MB drains through both DGE rings in parallel
                    eng, eng2 = ((nc.sync, nc.gpsimd) if t % 2 == 1
                                 else (nc.gpsimd, nc.sync))
                    eng.dma_start(out[t * 128:(t + 1) * 128, 0:2048],
                                  exp_t[:, 0:2048])
                    for ch in range(2, 4):
                        sl = slice(ch * 1024, (ch + 1) * 1024)
                        nc.vector.scalar_tensor_tensor(
                            exp_t[:, sl], exp_t[:, sl], rcol[:],
                            g1m_bc[:, sl], op0=ALU.mult, op1=ALU.mult)
                        if t == 15:
                            # final tile: quarter DMAs to shorten the tail
                            e = eng if ch == 2 else eng2
                            e.dma_start(out[t * 128:(t + 1) * 128, sl],
                                        exp_t[:, sl])
                    if t < 15:
                        eng2.dma_start(out[t * 128:(t + 1) * 128, 2048:4096],
                                       exp_t[:, 2048:4096])
    nc.compile()
    return nc


def kernel(x, Wq, bq, Wk, bk, Wv, bv, _trace=False, _tmpdir=None):
    x = np.asarray(x, dtype=np.float32)
    if "nc" not in _CACHE:
        _CACHE["nc"] = _build()
    nc = _CACHE["nc"]

    bv_f = np.float32(np.asarray(bv).reshape(())[()])
    eye_np = np.eye(128, dtype=np.float32)
    misc = np.zeros((128, 18), dtype=np.float32)
    misc[:, 0:4] = np.asarray(bq, np.float32).reshape(4, 128).T
    misc[:, 4:8] = np.asarray(bk, np.float32).reshape(4, 128).T
    wv_c = np.asarray(Wv, np.float32).reshape(4, 128).T
    misc[:, 8:16:2] = wv_c
    misc[:, 9:16:2] = wv_c
    misc[:, 16] = bv_f
    misc[:, 17] = -bv_f
    wq_np = np.ascontiguousarray(np.asarray(Wq, np.float32).astype(np.float16))
    wk_np = np.ascontiguousarray(np.asarray(Wk, np.float32).astype(np.float16))

    in_maps = []
    for c in range(8):
        b, h = c // 2, c % 2
        xT = x[b].T.astype(np.float16)  # (H, N)
        if h == 0:
            xqc = np.ascontiguousarray(xT)
        else:
            xqc = np.ascontiguousarray(
                np.concatenate([xT[:, NSH:], xT[:, :NSH]], axis=1))
        in_maps.append({"xq": xqc, "wq": wq_np, "wk": wk_np, "misc": misc,
                        "eye": eye_np})

    res = run_bass_kernel_spmd(nc, in_maps, list(range(8)), trace=_trace,
                               tmpdir=_tmpdir)

    outp = np.empty((B, N, N), dtype=np.float32)
    for c in range(8):
        b, h = c // 2, c % 2
        O = np.asarray(res.results[c]["out"]).astype(np.float32).T  # (i_perm, j)
        js = slice(h * NSH, (h + 1) * NSH)
        outp[b, h * NSH:(h + 1) * NSH, js] = O[:NSH]
        outp[b, (1 - h) * NSH:(2 - h) * NSH, js] = O[NSH:]
    if _trace:
        return outp, res
    return outp
